# revision 11
# baseline (speedup 1.0000x reference)
"""GCNConv (aggregate in 128ch, then x@W) + PReLU, distributed over 8 TRN2 NeuronCores.

Decomposition (exactly matches the reference):
    deg[v]  = in-degree of v including self-loop
    dinv    = deg ** -0.5
    xs[u]   = dinv[u] * x[u]                       (device, per-node scale)
    raw[v]  = sum_{e: dst=v} xs[src_e] + xs[v]     (gather + one-hot matmul agg)
    out     = PReLU(dinv[v] * (raw @ W) + b)       (dinv folded into epilogue)

Sharding: nodes are split contiguously over 8 cores (dst-ownership). Edges are
routed (host side) to the core owning their destination. Each core:
  1. scales its x shard -> xs shard, AllGather -> full xs (bf16) in local DRAM
  2. dma_gather's xs[src] for its edges (int16 idxs => 4 base-offset chunks)
  3. one-hot S matrices (built on DVE from iota==dst_rel) turn the segment sum
     into TensorEngine matmuls accumulating per-128-node-window PSUM tiles
  4. window epilogue: agg @ W + b (matmul), dinv scale + PReLU (ACT/DVE),
     DMA out its [12500, 512] output shard.
SPMD requires an identical instruction stream on all cores, so per-
(chunk,window) edge-group sizes are padded to the max over cores (pad edges
gather row 0 with dst_rel=-1 so their one-hot row is all zero).
"""

import math

import numpy as np

# Problem constants (hardcoded per the task spec).
N_NODES = 100000
N_EDGES = 1600000
IN_CH = 128
HID_CH = 512
N_CORES = 8

P = 128  # partitions / window size


class Cfg:
    def __init__(self, n_nodes, in_ch, hid_ch, n_cores, chunk, sww):
        assert n_nodes % n_cores == 0
        self.n = n_nodes
        self.in_ch = in_ch
        self.hid = hid_ch
        self.cores = n_cores
        self.nsh = n_nodes // n_cores  # nodes per core
        self.nw = math.ceil(self.nsh / P)  # 128-node windows per core
        self.chunk = chunk  # gather chunk rows (int16 idx range)
        self.nchunk = math.ceil(n_nodes / chunk)
        self.sww = sww  # windows per super-window (gather granularity)
        self.nsw = math.ceil(self.nw / sww)


CFG = Cfg(N_NODES, IN_CH, HID_CH, N_CORES, chunk=25000, sww=4)


def route(edge_index, cfg):
    """Host-side edge routing. Returns (dinv, per_core_arrays, structure).

    structure is identical for all cores (SPMD): a list of gather calls
    (sw, ch, n_idxs, idx_col_off, blk_off) plus per-window block layout
    win_blocks[w] = list of (ch, blk_off, nblk) in stream-block numbering.
    """
    src = np.asarray(edge_index[0]).astype(np.int64)
    dst = np.asarray(edge_index[1]).astype(np.int64)

    deg = np.bincount(dst, minlength=cfg.n).astype(np.float64) + 1.0
    dinv = (1.0 / np.sqrt(deg)).astype(np.float32)

    core = dst // cfg.nsh
    # group id within a core: g = ch * nw + w  (windows ordered inside chunks;
    # chunk-major => within (sw, ch) the windows are contiguous)
    per_core = []
    counts = np.zeros((cfg.cores, cfg.nchunk * cfg.nw), dtype=np.int64)
    for c in range(cfg.cores):
        m = core == c
        s = src[m]
        d = dst[m] - c * cfg.nsh
        w = d >> 7
        ch = s // cfg.chunk
        sw = w // cfg.sww
        g = (sw * cfg.nchunk + ch) * cfg.sww + (w - sw * cfg.sww)
        counts[c] = np.bincount(
            ch * cfg.nw + w, minlength=cfg.nchunk * cfg.nw
        )
        per_core.append((s, d, w, ch, g))

    padded = (
        np.ceil(np.max(counts, axis=0) / P).astype(np.int64) * P
    )  # [nchunk*nw], multiple of 128

    # stream order: (sw, ch, w) ; slot for group (ch, w) has size padded[ch*nw+w]
    order = []  # group ids in stream order
    for sw in range(cfg.nsw):
        ws = range(sw * cfg.sww, min((sw + 1) * cfg.sww, cfg.nw))
        for ch in range(cfg.nchunk):
            for w in ws:
                order.append(ch * cfg.nw + w)
    order = np.array(order, dtype=np.int64)
    sizes = padded[order]
    starts = np.zeros_like(sizes)
    starts[1:] = np.cumsum(sizes)[:-1]
    total = int(sizes.sum())
    group_start = np.zeros(cfg.nchunk * cfg.nw, dtype=np.int64)
    group_start[order] = starts

    # structure: gather calls + per-window block layout
    calls = []
    win_blocks = [[] for _ in range(cfg.nw)]
    pos = 0
    for sw in range(cfg.nsw):
        ws = range(sw * cfg.sww, min((sw + 1) * cfg.sww, cfg.nw))
        for ch in range(cfg.nchunk):
            call_n = int(sum(padded[ch * cfg.nw + w] for w in ws))
            if call_n > 0:
                calls.append(
                    dict(sw=sw, ch=ch, n=call_n, off=pos)
                )  # off = edge offset in stream
            p2 = pos
            for w in ws:
                nb = int(padded[ch * cfg.nw + w]) // P
                if nb > 0:
                    win_blocks[w].append((ch, p2 // P, nb))
                p2 += int(padded[ch * cfg.nw + w])
            pos = p2
    assert pos == total

    # per-core idx / dst_rel arrays
    core_arrays = []
    for c in range(cfg.cores):
        s, d, w, ch, g = per_core[c]
        idx = np.zeros(total, dtype=np.int16)
        rel = np.full(total, -1.0, dtype=np.float32)
        dvd = np.zeros(total, dtype=np.float32)
        # scatter edges into their group slots
        ordr = np.argsort(ch * cfg.nw + w, kind="stable")
        gsorted = (ch * cfg.nw + w)[ordr]
        # rank within group
        grp_first = np.searchsorted(gsorted, np.arange(cfg.nchunk * cfg.nw), "left")
        rank = np.arange(len(gsorted)) - grp_first[gsorted]
        slot = group_start[gsorted] + rank
        idx[slot] = (s[ordr] - ch[ordr] * cfg.chunk).astype(np.int16)
        rel[slot] = (d[ordr] - w[ordr] * P).astype(np.float32)
        dvd[slot] = dinv[c * cfg.nsh + d[ordr]]
        # layouts: idx wrapped [16, total/16] replicated to 128 partitions;
        # rel as [128, total/128] (edge j -> [j%128, j//128])
        idx_w = np.tile(idx.reshape(total // 16, 16).T, (8, 1)).copy()
        rel_w = rel.reshape(total // P, P).T.copy()
        dvd_w = dvd.reshape(total // P, P).T.copy()
        core_arrays.append((idx_w, rel_w, dvd_w))

    return dinv, core_arrays, dict(calls=calls, win_blocks=win_blocks, total=total)


def build(cfg, structure):
    import concourse.bass as bass
    import concourse.tile as tile
    from concourse import bacc, mybir

    f32 = mybir.dt.float32
    bf16 = mybir.dt.bfloat16
    i16 = mybir.dt.int16
    i32 = mybir.dt.int32
    AF = mybir.ActivationFunctionType
    OP = mybir.AluOpType

    calls = structure["calls"]
    win_blocks = structure["win_blocks"]
    total = structure["total"]

    nsh, nw, hid, in_ch = cfg.nsh, cfg.nw, cfg.hid, cfg.in_ch
    last_rows = nsh - (nw - 1) * P  # rows in the final (partial) window

    nc = bacc.Bacc(
        "TRN2", target_bir_lowering=False, debug=False, num_devices=cfg.cores
    )

    x_sh = nc.declare_dram_parameter("x_sh", [nsh, in_ch], f32, isOutput=False)
    dinv_t = nc.declare_dram_parameter("dinv_t", [P, nw], f32, isOutput=False)
    w_p = nc.declare_dram_parameter("w_p", [in_ch, hid], f32, isOutput=False)
    b_p = nc.declare_dram_parameter("b_p", [1, hid], f32, isOutput=False)
    alpha_p = nc.declare_dram_parameter("alpha_p", [1, 1], f32, isOutput=False)
    idx_p = nc.declare_dram_parameter("idx_p", [P, total // 16], i16, isOutput=False)
    rel_p = nc.declare_dram_parameter("rel_p", [P, total // P], f32, isOutput=False)
    dvd_p = nc.declare_dram_parameter("dvd_p", [P, total // P], f32, isOutput=False)
    out_p = nc.declare_dram_parameter("out", [nsh, hid], f32, isOutput=True)

    xs_loc = nc.dram_tensor("xs_loc", [nsh, in_ch], bf16)
    xs_full = nc.dram_tensor("xs_full", [cfg.n, in_ch], bf16, addr_space="Shared")

    with tile.TileContext(nc) as tc:
        with (
            tc.tile_pool(name="const", bufs=1) as constp,
            tc.tile_pool(name="xsbuf", bufs=1) as xsp,
            tc.tile_pool(name="xin", bufs=3) as xinp,
            tc.tile_pool(name="gath", bufs=5) as gp,
            tc.tile_pool(name="smat", bufs=8) as sp,
            tc.tile_pool(name="idxs", bufs=5) as idxp,
            tc.tile_pool(name="aggt", bufs=3) as aggp,
            tc.tile_pool(name="epi", bufs=3) as epip,
            tc.tile_pool(name="psw", bufs=4, space="PSUM") as pswp,
            tc.tile_pool(name="pso", bufs=2, space="PSUM") as psop,
            tc.tile_pool(name="psa", bufs=1, space="PSUM") as psap,
        ):
            # ---- constants / setup ----
            iota_i = constp.tile([P, P], i32)
            nc.gpsimd.iota(iota_i[:], pattern=[[1, P]], base=0, channel_multiplier=0)
            iota_f = constp.tile([P, P], f32)
            nc.vector.tensor_copy(iota_f[:], iota_i[:])
            lane_i = constp.tile([P, 1], i32)
            nc.gpsimd.iota(lane_i[:], pattern=[[1, 1]], base=0, channel_multiplier=1)
            lane_f = constp.tile([P, 1], f32)
            nc.vector.tensor_copy(lane_f[:], lane_i[:])
            ident_bf = constp.tile([P, P], bf16)
            nc.vector.tensor_scalar(
                ident_bf[:], iota_f[:], lane_f[:], None, OP.is_equal
            )

            w_f32 = constp.tile([in_ch, hid], f32)
            nc.sync.dma_start(w_f32[:], w_p[:])
            w_bf = constp.tile([in_ch, hid], bf16)
            nc.vector.tensor_copy(w_bf[:], w_f32[:])

            b_sb = constp.tile([1, hid], f32)
            nc.sync.dma_start(b_sb[:], b_p[:])
            ones1 = constp.tile([1, P], f32)
            nc.vector.memset(ones1[:], 1.0)
            alpha_sb = constp.tile([1, 1], f32)
            nc.sync.dma_start(alpha_sb[:], alpha_p[:])

            dinv_sb = constp.tile([P, nw], f32)
            nc.sync.dma_start(dinv_sb[:], dinv_t[:])

            # alpha broadcast to [128,1] via K=1 matmul with ones
            psum_a = psap.tile([P, 1], f32, space="PSUM")
            nc.tensor.matmul(
                psum_a[:], lhsT=ones1[:], rhs=alpha_sb[:], start=True, stop=True
            )
            alpha_bc = constp.tile([P, 1], f32)
            nc.vector.tensor_copy(alpha_bc[:], psum_a[:])

            # ---- phase 1: xs = dinv * x ; AllGather ----
            xs_sb = xsp.tile([P, nw, in_ch], bf16)
            for w in range(nw):
                rows = P if w < nw - 1 else last_rows
                xt = xinp.tile([P, in_ch], f32, tag="xt")
                if rows < P:
                    nc.vector.memset(xt[:], 0.0)
                nc.sync.dma_start(xt[:rows, :], x_sh[w * P : w * P + rows, :])
                nc.vector.tensor_scalar(
                    xs_sb[:, w, :], xt[:], dinv_sb[:, w : w + 1], None, OP.mult
                )
                nc.sync.dma_start(
                    xs_loc[w * P : w * P + rows, :], xs_sb[:rows, w, :]
                )

            nc.gpsimd.collective_compute(
                "AllGather",
                mybir.AluOpType.bypass,
                replica_groups=[list(range(cfg.cores))],
                ins=[xs_loc[:]],
                outs=[xs_full[:]],
            )

            # ---- phase 2: gather + aggregate + epilogue ----
            # pre-create per-(sw,ch) gather + S tiles on demand
            call_by_swch = {(cl["sw"], cl["ch"]): cl for cl in calls}
            g_tiles = {}
            s_tiles = {}

            def emit_call(cl):
                n = cl["n"]
                nb = n // P
                gt = gp.tile([P, nb, in_ch], bf16, tag="g")
                it = idxp.tile([P, n // 16], i16, tag="idx")
                nc.sync.dma_start(
                    it[:], idx_p[:, cl["off"] // 16 : (cl["off"] + n) // 16]
                )
                ch0 = cl["ch"] * cfg.chunk
                ch1 = min(ch0 + cfg.chunk, cfg.n)
                nc.gpsimd.dma_gather(
                    gt[:],
                    xs_full[ch0:ch1, :],
                    it[:],
                    n,
                    n,
                    in_ch,
                    single_packet=False,
                )
                # one-hot S for the whole call
                st = sp.tile([P, nb * P], bf16, tag="s")
                s_tiles[(cl["sw"], cl["ch"])] = (st, cl)
                g_tiles[(cl["sw"], cl["ch"])] = (gt, cl)
                # build S per block: (iota == dst_rel) * dinv[dst]
                rel_sb = idxp.tile([P, nb], f32, tag="rel")
                nc.sync.dma_start(
                    rel_sb[:], rel_p[:, cl["off"] // P : cl["off"] // P + nb]
                )
                dvd_sb = idxp.tile([P, nb], f32, tag="dvd")
                nc.sync.dma_start(
                    dvd_sb[:], dvd_p[:, cl["off"] // P : cl["off"] // P + nb]
                )
                for bi in range(nb):
                    nc.vector.tensor_scalar(
                        st[:, bi * P : (bi + 1) * P],
                        iota_f[:],
                        rel_sb[:, bi : bi + 1],
                        dvd_sb[:, bi : bi + 1],
                        OP.is_equal,
                        OP.mult,
                    )

            for sw in range(cfg.nsw):
                for ch in range(cfg.nchunk):
                    cl = call_by_swch.get((sw, ch))
                    if cl is not None:
                        emit_call(cl)
                ws = range(sw * cfg.sww, min((sw + 1) * cfg.sww, cfg.nw))
                for w in ws:
                    rows = P if w < nw - 1 else last_rows
                    psw = pswp.tile([P, P], f32, space="PSUM", tag="psw")
                    # self-loop injection: psw = xs_win^T @ diag(dinv_win)
                    diag_w = aggp.tile([P, P], bf16, tag="diag")
                    nc.vector.tensor_scalar(
                        diag_w[:], ident_bf[:], dinv_sb[:, w : w + 1], None, OP.mult
                    )
                    blocks = win_blocks[w]
                    nc.tensor.matmul(
                        psw[:],
                        lhsT=xs_sb[:, w, :],
                        rhs=diag_w[:],
                        start=True,
                        stop=(len(blocks) == 0),
                    )
                    nmm = sum(nb for _, _, nb in blocks)
                    k = 0
                    for ch, boff, nb in blocks:
                        gt, cl = g_tiles[(sw, ch)]
                        st, _ = s_tiles[(sw, ch)]
                        local = boff - cl["off"] // P
                        for bi in range(nb):
                            k += 1
                            nc.tensor.matmul(
                                psw[:],
                                lhsT=gt[:, local + bi, :],
                                rhs=st[:, (local + bi) * P : (local + bi + 1) * P],
                                start=False,
                                stop=(k == nmm),
                            )
                    # epilogue
                    aggt = aggp.tile([P, P], bf16, tag="aggt")
                    nc.vector.tensor_copy(aggt[:], psw[:])
                    pso = psop.tile([P, hid], f32, space="PSUM", tag="pso")
                    nc.tensor.matmul(
                        pso[:], lhsT=ones1[:], rhs=b_sb[:], start=True, stop=False
                    )
                    nc.tensor.matmul(
                        pso[:], lhsT=aggt[:], rhs=w_bf[:], start=False, stop=True
                    )
                    pos = epip.tile([P, hid], f32, tag="pos")
                    nc.scalar.activation(pos[:], pso[:], AF.Relu)
                    neg = epip.tile([P, hid], f32, tag="neg")
                    nc.vector.tensor_scalar(
                        neg[:],
                        pso[:],
                        alpha_bc[:, :1],
                        0.0,
                        OP.mult,
                        OP.min,
                    )
                    ot = epip.tile([P, hid], f32, tag="ot")
                    nc.vector.tensor_tensor(ot[:], pos[:], neg[:], op=OP.add)
                    nc.sync.dma_start(
                        out_p[w * P : w * P + rows, :], ot[:rows, :]
                    )

    nc.compile()
    return nc


def _prep_inputs(x, edge_index, W, b, alpha, cfg):
    dinv, core_arrays, structure = route(edge_index, cfg)
    x = np.asarray(x, dtype=np.float32)
    W = np.asarray(W, dtype=np.float32)
    b = np.asarray(b, dtype=np.float32).reshape(1, cfg.hid)
    alpha = np.asarray(alpha, dtype=np.float32).reshape(1, 1)

    pad_n = cfg.nw * P - cfg.nsh
    in_maps = []
    for c in range(cfg.cores):
        idx_w, rel_w, dvd_w = core_arrays[c]
        dsh = dinv[c * cfg.nsh : (c + 1) * cfg.nsh]
        dsh = np.concatenate([dsh, np.ones(pad_n, np.float32)])
        in_maps.append(
            {
                "x_sh": x[c * cfg.nsh : (c + 1) * cfg.nsh],
                "dinv_t": dsh.reshape(cfg.nw, P).T.copy(),
                "w_p": W,
                "b_p": b,
                "alpha_p": alpha,
                "idx_p": idx_w,
                "rel_p": rel_w,
                "dvd_p": dvd_w,
            }
        )
    return in_maps, structure


def kernel(x, edge_index, W, b, alpha):
    from concourse.bass_utils import run_bass_kernel_spmd

    cfg = CFG
    in_maps, structure = _prep_inputs(x, edge_index, W, b, alpha, cfg)
    nc = build(cfg, structure)
    res = run_bass_kernel_spmd(nc, in_maps, list(range(cfg.cores)))
    out = np.concatenate(
        [np.asarray(res.results[c]["out"]) for c in range(cfg.cores)], axis=0
    )
    return out.astype(np.float32)
